# revision 1
# baseline (speedup 1.0000x reference)
"""Bass/Trainium2 kernel for nn_F_Loss_65446711656630.

Strategy (data-parallel over N, 8 cores):
  - Host: GLOBAL stable sort of all rows by class id, then slice 8192 rows
    per core and transpose to [512 features x 8192 rows] contiguous pieces.
    After a global sort each core spans only ~2 classes, so class
    transitions are rare at any granularity.
  - Device (static kernel): stream 16 fp16 pieces of [128, 2048]; per piece
      * DVE:  per-128-row-chunk partial sums of h (one multi-chunk
              TensorReduce per piece, 3D access pattern)
      * ACT:  square with accum_out -> per-piece partial sum of h^2
              (the square pass itself yields the sumsq reduction, so no
              second DVE reduce pass is needed)
    Inputs ship as fp16 (costs ~1e-5 final rel err, halves HBM traffic to
    8 MiB/core); the per-element engine passes (~36-39 us) are the cap,
    with DMA at ~23 us well underneath.
  - Host: per-class stats from single-class chunk/piece partials (fp64)
    + direct numpy sums for the few transition chunks/pieces; then the
    tiny O(C^2 D) pairwise betainc/top-k stage in f32 jax on CPU
    (mirroring the reference's numerics exactly).
"""

import numpy as np

C = 16
D = 512
N = 65536
NCORES = 8
ROWS = N // NCORES          # 8192 rows per core
P = 128                     # SBUF partitions
PIECE = 2048                # rows per DMA piece / sumsq granule
X = 128                     # rows per sums granule (DVE reduce chunk)
NBLK = D // P               # 4 feature blocks
NPIECE = ROWS // PIECE      # 4 pieces per block
NCHUNK = ROWS // X          # 64 chunks per core
CPP = PIECE // X            # 8 chunks per piece
XMIN, XMAX = 1e-37, 1.0 - 1e-5

_NC_CACHE = {}


def _build_nc():
    """Per-core SPMD program.

    Inputs:  "ht"   [16, 128, 2048] fp16 (piece (b,p) at index b*4+p holds
                                         features b*128..+128 x rows
                                         p*2048..+2048, contiguous)
    Outputs: "hsum" [128, 256] f32  (hsum[f, b*64+g] = sum over 128-row
                                     chunk g of feature b*128+f)
             "ssum" [128, 16]  f32  (ssum[f, b*4+p]  = sum over piece p's
                                     2048 rows of feature (b*128+f)^2)
    """
    import concourse.tile as tile
    from concourse import bacc, mybir

    f32 = mybir.dt.float32

    nc = bacc.Bacc("TRN2", target_bir_lowering=False, debug=False,
                   num_devices=NCORES)
    f16 = mybir.dt.float16
    ht = nc.declare_dram_parameter("ht", [NBLK * NPIECE, P, PIECE], f16,
                                   isOutput=False)
    hsum = nc.declare_dram_parameter("hsum", [P, NBLK * NCHUNK], f32, isOutput=True)
    ssum = nc.declare_dram_parameter("ssum", [P, NBLK * NPIECE], f32, isOutput=True)

    with tile.TileContext(nc) as tc:
        with (
            tc.tile_pool(name="pc", bufs=8) as piece_pool,
            tc.tile_pool(name="sq", bufs=3) as sq_pool,
            tc.tile_pool(name="acc", bufs=1) as acc_pool,
        ):
            hpart = acc_pool.tile([P, NBLK * NCHUNK], f32, tag="hpart")
            spart = acc_pool.tile([P, NBLK * NPIECE], f32, tag="spart")

            for i in range(NBLK * NPIECE):
                t = piece_pool.tile([P, PIECE], f16)
                nc.sync.dma_start(t[:], ht[i])

                # ACT: square (scratch) + free-dim accumulate -> piece sumsq
                sq = sq_pool.tile([P, PIECE], f32)
                nc.scalar.activation(
                    sq[:], t[:], mybir.ActivationFunctionType.Square,
                    accum_out=spart[:, i:i + 1])

                # DVE: one multi-chunk reduce -> chunk sums of h
                base = i * CPP
                t3 = t[:].rearrange("p (c x) -> p c x", x=X)
                nc.vector.reduce_sum(
                    hpart[:, base:base + CPP], t3, axis=mybir.AxisListType.X)

            nc.sync.dma_start(hsum[:], hpart[:])
            nc.sync.dma_start(ssum[:], spart[:])
    nc.compile()
    return nc


def _get_nc():
    if "nc" not in _NC_CACHE:
        _NC_CACHE["nc"] = _build_nc()
    return _NC_CACHE["nc"]


def _granule_classes(ids_sorted, size):
    """Per-granule class id, or -1 if the granule spans a class boundary."""
    g = ids_sorted.reshape(-1, size)
    pure = g[:, 0] == g[:, -1]
    return np.where(pure, g[:, 0], -1).astype(np.int64)


def _prep_core(hs_k, ids_k):
    """hs_k/ids_k already globally sorted. Returns device input + host fixups."""
    T = np.ascontiguousarray(
        hs_k.reshape(NPIECE, PIECE, NBLK, P).transpose(2, 0, 3, 1)
        .astype(np.float16)
    ).reshape(NBLK * NPIECE, P, PIECE)           # [16, 128, 2048] fp16

    chunk_cls = _granule_classes(ids_k, X)       # [64]
    piece_cls = _granule_classes(ids_k, PIECE)   # [8]

    bsum = np.zeros((C, D), dtype=np.float64)
    bsq = np.zeros((C, D), dtype=np.float64)
    # transition chunks: host computes their per-class h sums
    if (chunk_cls < 0).any():
        m = np.repeat(chunk_cls < 0, X)
        rows, rids = hs_k[m].astype(np.float64), ids_k[m]
        for q in np.unique(rids):
            bsum[q] += rows[rids == q].sum(axis=0)
    # transition pieces: host computes their per-class h^2 sums
    if (piece_cls < 0).any():
        m = np.repeat(piece_cls < 0, PIECE)
        rows, rids = hs_k[m].astype(np.float64), ids_k[m]
        for q in np.unique(rids):
            sel = rows[rids == q]
            bsq[q] += (sel * sel).sum(axis=0)
    return T, chunk_cls, piece_cls, bsum, bsq


def _device_stats(hidden, ids, **run_kwargs):
    """Returns (sums[C,D], sumsq[C,D]) float64, plus the raw run result."""
    from concourse import bass_utils

    nc = _get_nc()

    order = np.argsort(ids, kind="stable")       # GLOBAL sort by class
    ids_s = ids[order]
    hs = hidden[order]

    in_maps = []
    meta = []
    sums = np.zeros((C, D), dtype=np.float64)
    sumsq = np.zeros((C, D), dtype=np.float64)
    for k in range(NCORES):
        rows = slice(k * ROWS, (k + 1) * ROWS)
        T, ccls, pcls, bsum, bsq = _prep_core(hs[rows], ids_s[rows])
        in_maps.append({"ht": T})
        meta.append((ccls, pcls))
        sums += bsum
        sumsq += bsq

    res = bass_utils.run_bass_kernel_spmd(nc, in_maps, list(range(NCORES)), **run_kwargs)

    eye = np.arange(C)[None, :]
    for k in range(NCORES):
        ccls, pcls = meta[k]
        hp = res.results[k]["hsum"].astype(np.float64)
        sp = res.results[k]["ssum"].astype(np.float64)
        # [128, b, g] -> [g, b, 128] -> [granule, feature]
        hp = hp.reshape(P, NBLK, NCHUNK).transpose(2, 1, 0).reshape(NCHUNK, D)
        sp = sp.reshape(P, NBLK, NPIECE).transpose(2, 1, 0).reshape(NPIECE, D)
        cm = ccls >= 0
        sums += ((ccls[cm, None] == eye).astype(np.float64)).T @ hp[cm]
        pm = pcls >= 0
        sumsq += ((pcls[pm, None] == eye).astype(np.float64)).T @ sp[pm]
    return sums, sumsq, res


def _pairwise_loss(counts, sums, sumsq, d):
    """The tiny O(C^2 D) stage on host CPU.

    Runs in float32 with the same jax ops as the reference: at these extreme
    betainc parameters (b ~ 8190, x ~ 1e-5) jax's f32 betainc differs from
    the true (f64) value by ~1e-3, so matching the reference requires
    replicating its f32 numerics, not improving on them.
    """
    import jax
    import jax.numpy as jnp

    cpu = jax.devices("cpu")[0]
    with jax.default_device(cpu):
        counts64 = counts.astype(np.float64)
        means64 = sums / counts64[:, None]
        withins64 = sumsq - counts64[:, None] * means64**2
        counts = jnp.asarray(counts64, jnp.float32)               # [C]
        means = jnp.asarray(means64, jnp.float32)                 # [C, D]
        withins = jnp.asarray(withins64, jnp.float32)             # [C, D]
        half_diff = (means[:, None, :] - means[None, :, :]) * 0.5
        pair_counts = counts[:, None] + counts[None, :]
        pair_between = half_diff * half_diff * pair_counts[:, :, None]
        pair_within = withins[:, None, :] + withins[None, :, :]
        d2 = pair_counts - 2.0
        d2 = jnp.where(d2 == 0.0, 1e-5, d2)
        x = pair_between / (pair_between + pair_within)
        x = jnp.clip(x, XMIN, XMAX)
        a = jnp.full_like(x, 0.5)
        b = jnp.broadcast_to((d2 * 0.5)[:, :, None], x.shape)
        xbetainc = jax.scipy.special.betainc(a, b, x)             # [C, C, D]
        top_k, _ = jax.lax.top_k(xbetainc, int(d))                # [C, C, d]
        per_pair = jnp.sum(jnp.log(top_k), axis=-1)               # [C, C]
        mask = jnp.triu(jnp.ones((C, C), dtype=bool), k=1)
        total = jnp.sum(jnp.where(mask, per_pair, jnp.zeros_like(per_pair)))
        return float(-total)


def kernel(hidden, batch_ids, d):
    hidden = np.asarray(hidden, dtype=np.float32)
    ids = np.asarray(batch_ids).astype(np.int64)
    assert hidden.shape == (N, D), hidden.shape

    counts = np.bincount(ids, minlength=C).astype(np.float64)
    sums, sumsq, _ = _device_stats(hidden, ids)
    total = _pairwise_loss(counts, sums, sumsq, int(np.asarray(d)))
    return np.array(total, dtype=np.float32)



# revision 10
# speedup vs baseline: 1.0305x; 1.0305x over previous
"""Bass/Trainium2 kernel for nn_F_Loss_65446711656630.

Strategy (data-parallel over N, 8 cores):
  - Host: per core slice 8192 rows, lay out as 16 pieces of
    [128 rows x 2048 (= 4 chunks x 512 feats)] fp16 (rows on partitions),
    plus a per-chunk one-hot class matrix W [128 rows, 64 chunks * 16
    classes] fp16.  No sorting, no boundary fixups: the one-hot IS the
    segment assignment.
  - Device: segment-sum as matmul on the (otherwise idle) TensorEngine:
    for each 128-row chunk k,  psum[16, 512] += W_k^T @ chunk  gives the
    per-class sums directly.  A second accumulator takes W_k^T @ chunk^2
    for the per-class sum-of-squares; the element-wise square pass is
    split between DVE (tensor_tensor mult, 2x fp16 mode) and ACT
    (Square activation, 1x) so both stay under the DMA time.  Four PSUM
    accumulators (h/sq x even/odd chunks) live in four separate banks at
    column-group offsets 0/32/64/96 so consecutive matmuls target
    different PE column groups and overlap.
  - Host: add the 4 accumulators (fp64), then the tiny O(C^2 D) pairwise
    betainc/top-k stage in f32 jax on CPU (mirroring the reference's
    numerics exactly).
"""

import numpy as np

C = 16
D = 512
N = 65536
NCORES = 8
ROWS = N // NCORES          # 8192 rows per core
P = 128                     # SBUF partitions / rows per chunk
CHUNKS = ROWS // P          # 64 chunks per core
CPP = 4                     # chunks per DMA piece
PIECE_COLS = CPP * D        # 2048
NPIECE = CHUNKS // CPP      # 16 pieces per core
DVE_PIECES = 11             # pieces squared on DVE; the rest on ACT
XMIN, XMAX = 1e-37, 1.0 - 1e-5

_NC_CACHE = {}


def _build_nc():
    """Per-core SPMD program.

    Inputs:  "ht" [16, 128, 2048] fp16  (piece p, partition r, col c*512+f
                                         = hidden[(4p+c)*128 + r, f])
             "w"  [128, 1024] fp16      (w[r, k*16+q] = 1 iff row r of
                                         chunk k has class q)
    Output:  "ob" [128, 512] f32        (partitions 32g..32g+16 hold group
                                         g: 0 h-sums even chunks, 1 h-sums
                                         odd, 2 sq-sums even, 3 sq-sums odd)
    """
    import concourse.tile as tile
    from concourse import bacc, mybir

    f32 = mybir.dt.float32
    f16 = mybir.dt.float16

    nc = bacc.Bacc("TRN2", target_bir_lowering=False, debug=False,
                   num_devices=NCORES)
    ht = nc.declare_dram_parameter("ht", [NPIECE, P, PIECE_COLS], f16,
                                   isOutput=False)
    w = nc.declare_dram_parameter("w", [P, CHUNKS * C], f16, isOutput=False)
    ob = nc.declare_dram_parameter("ob", [P, D], f32, isOutput=True)

    with tile.TileContext(nc) as tc:
        with (
            tc.tile_pool(name="pc", bufs=6) as piece_pool,
            tc.tile_pool(name="sq", bufs=4) as sq_pool,
            tc.tile_pool(name="wp", bufs=1) as w_pool,
            tc.psum_pool(name="ps", bufs=1) as psum_pool,
        ):
            wsb = w_pool.tile([P, CHUNKS * C], f16, tag="wsb")
            nc.sync.dma_start(wsb[:], w[:])

            # one accumulator bank per (stat, parity) group, at column-group
            # offset 32*g so consecutive matmuls overlap on the PE array
            banks = [psum_pool.tile([P, D], f32, name=f"acc{g}",
                                    tag=f"acc{g}")
                     for g in range(4)]
            accs = [banks[g][32 * g:32 * g + C, :] for g in range(4)]
            started = [False] * 4
            n_issued = [0] * 4

            def seg_mm(g, k, rhs):
                first = not started[g]
                started[g] = True
                n_issued[g] += 1
                last = n_issued[g] == CHUNKS // 2
                nc.tensor.matmul(
                    accs[g], wsb[:, k * C:(k + 1) * C], rhs,
                    start=first, stop=last, tile_position=(0, 32 * g))

            sq_tiles = [None] * NPIECE

            def issue_sq_mms(p):
                t = sq_tiles[p]
                for c in range(CPP):
                    k = p * CPP + c
                    seg_mm(2 + (k % 2), k, t[:, c * D:(c + 1) * D])

            for p in range(NPIECE):
                t = piece_pool.tile([P, PIECE_COLS], f16)
                nc.sync.dma_start(t[:], ht[p])

                sq = sq_pool.tile([P, PIECE_COLS], f16)
                if p < DVE_PIECES:
                    nc.vector.tensor_mul(sq[:], t[:], t[:])
                else:
                    nc.scalar.square(sq[:], t[:])
                sq_tiles[p] = sq

                # h matmuls for piece p; sq matmuls for piece p-1 (ready)
                for c in range(CPP):
                    k = p * CPP + c
                    seg_mm(k % 2, k, t[:, c * D:(c + 1) * D])
                if p > 0:
                    issue_sq_mms(p - 1)
            issue_sq_mms(NPIECE - 1)

            ob_sb = w_pool.tile([P, D], f32, tag="ob_sb")
            for g in range(4):
                if g % 2 == 0:
                    nc.vector.tensor_copy(ob_sb[32 * g:32 * g + C, :], accs[g])
                else:
                    nc.scalar.copy(ob_sb[32 * g:32 * g + C, :], accs[g])
            nc.sync.dma_start(ob[:], ob_sb[:])
    nc.compile()
    return nc


def _get_nc():
    if "nc" not in _NC_CACHE:
        _NC_CACHE["nc"] = _build_nc()
    return _NC_CACHE["nc"]


def _prep_core(h_k, ids_k):
    T = np.ascontiguousarray(
        h_k.reshape(NPIECE, CPP, P, D).transpose(0, 2, 1, 3)
        .astype(np.float16)
    ).reshape(NPIECE, P, PIECE_COLS)

    ids2 = ids_k.reshape(CHUNKS, P)
    W3 = np.zeros((P, CHUNKS, C), dtype=np.float16)
    k_idx = np.broadcast_to(np.arange(CHUNKS)[:, None], (CHUNKS, P))
    r_idx = np.broadcast_to(np.arange(P)[None, :], (CHUNKS, P))
    W3[r_idx, k_idx, ids2] = 1.0
    return T, W3.reshape(P, CHUNKS * C)


def _device_stats(hidden, ids, **run_kwargs):
    """Returns (sums[C,D], sumsq[C,D]) float64, plus the raw run result."""
    from concourse import bass_utils

    nc = _get_nc()

    in_maps = []
    for k in range(NCORES):
        rows = slice(k * ROWS, (k + 1) * ROWS)
        T, W = _prep_core(hidden[rows], ids[rows])
        in_maps.append({"ht": T, "w": W})

    res = bass_utils.run_bass_kernel_spmd(nc, in_maps, list(range(NCORES)),
                                          **run_kwargs)

    sums = np.zeros((C, D), dtype=np.float64)
    sumsq = np.zeros((C, D), dtype=np.float64)
    for k in range(NCORES):
        ob = res.results[k]["ob"].astype(np.float64)
        sums += ob[0:C] + ob[32:32 + C]
        sumsq += ob[64:64 + C] + ob[96:96 + C]
    return sums, sumsq, res


def _pairwise_loss(counts, sums, sumsq, d):
    """The tiny O(C^2 D) stage on host CPU.

    Runs in float32 with the same jax ops as the reference: at these extreme
    betainc parameters (b ~ 8190, x ~ 1e-5) jax's f32 betainc differs from
    the true (f64) value by ~1e-3, so matching the reference requires
    replicating its f32 numerics, not improving on them.
    """
    import jax
    import jax.numpy as jnp

    cpu = jax.devices("cpu")[0]
    with jax.default_device(cpu):
        counts64 = counts.astype(np.float64)
        means64 = sums / counts64[:, None]
        withins64 = sumsq - counts64[:, None] * means64**2
        counts = jnp.asarray(counts64, jnp.float32)               # [C]
        means = jnp.asarray(means64, jnp.float32)                 # [C, D]
        withins = jnp.asarray(withins64, jnp.float32)             # [C, D]
        half_diff = (means[:, None, :] - means[None, :, :]) * 0.5
        pair_counts = counts[:, None] + counts[None, :]
        pair_between = half_diff * half_diff * pair_counts[:, :, None]
        pair_within = withins[:, None, :] + withins[None, :, :]
        d2 = pair_counts - 2.0
        d2 = jnp.where(d2 == 0.0, 1e-5, d2)
        x = pair_between / (pair_between + pair_within)
        x = jnp.clip(x, XMIN, XMAX)
        a = jnp.full_like(x, 0.5)
        b = jnp.broadcast_to((d2 * 0.5)[:, :, None], x.shape)
        xbetainc = jax.scipy.special.betainc(a, b, x)             # [C, C, D]
        top_k, _ = jax.lax.top_k(xbetainc, int(d))                # [C, C, d]
        per_pair = jnp.sum(jnp.log(top_k), axis=-1)               # [C, C]
        mask = jnp.triu(jnp.ones((C, C), dtype=bool), k=1)
        total = jnp.sum(jnp.where(mask, per_pair, jnp.zeros_like(per_pair)))
        return float(-total)


def kernel(hidden, batch_ids, d):
    hidden = np.asarray(hidden, dtype=np.float32)
    ids = np.asarray(batch_ids).astype(np.int64)
    assert hidden.shape == (N, D), hidden.shape

    counts = np.bincount(ids, minlength=C).astype(np.float64)
    sums, sumsq, _ = _device_stats(hidden, ids)
    total = _pairwise_loss(counts, sums, sumsq, int(np.asarray(d)))
    return np.array(total, dtype=np.float32)


# revision 11
# speedup vs baseline: 1.3665x; 1.3261x over previous
"""Bass/Trainium2 kernel for nn_F_Loss_65446711656630.

Strategy (data-parallel over N, 8 cores):
  - Host: per core slice 8192 rows, lay out as 16 pieces of
    [128 rows x 2048 (= 4 chunks x 512 feats)] (rows on partitions), in a
    mixed-precision schedule (10 pieces fp8e4m3 + 6 pieces fp16 -> ~16.3 us
    DMA stream vs 23.4 for all-fp16, while the square pass stays under the
    stream rate: fp8 squares run at 1x on DVE/ACT, fp16 at 2x on DVE).
    Plus per-chunk one-hot class matrices W [128 rows, 64 chunks * 16
    classes] in fp16 and fp8.  No sorting, no boundary fixups: the one-hot
    IS the segment assignment.
  - Device: segment-sum as matmul on the TensorEngine: for each 128-row
    chunk k,  psum[16, 512] += W_k^T @ chunk  gives the per-class sums
    directly; a second accumulator takes W_k^T @ square(chunk).  Squares
    (always fp16 out, for precision) are split DVE/ACT interleaved with
    piece arrival order so both engines run concurrently with the DMA
    stream.  Four PSUM accumulators (h/sq x even/odd chunks) live in four
    banks at column-group offsets 0/32/64/96 so consecutive matmuls target
    different PE column groups and overlap.
  - Host: add the 4 accumulators (fp64), then the tiny O(C^2 D) pairwise
    betainc/top-k stage in f32 jax on CPU (mirroring the reference's
    numerics exactly).
"""

import numpy as np

C = 16
D = 512
N = 65536
NCORES = 8
ROWS = N // NCORES          # 8192 rows per core
P = 128                     # SBUF partitions / rows per chunk
CHUNKS = ROWS // P          # 64 chunks per core
CPP = 4                     # chunks per DMA piece
PIECE_COLS = CPP * D        # 2048
NPIECE = CHUNKS // CPP      # 16 pieces per core
XMIN, XMAX = 1e-37, 1.0 - 1e-5

# square-pass schedule, interleaved with arrival order (piece index):
#   ACT squares 7 fp8 pieces (1x, ~2.3us each), DVE squares 3 fp8 (1x)
#   + 6 fp16 (2x, ~1.4us) -- both engines stay at/under the ~16us stream.
ACT_FP8 = (0, 2, 4, 7, 9, 12, 14)
DVE_FP8 = (3, 8, 13)
DVE_FP16 = (1, 5, 6, 10, 11, 15)
FP16_PIECES = frozenset(DVE_FP16)
FP8_PIECES = tuple(p for p in range(NPIECE) if p not in FP16_PIECES)

_NC_CACHE = {}


def _build_nc():
    """Per-core SPMD program.

    Inputs:  "ht16" [6, 128, 2048] fp16   (fp16 pieces, arrival-order slot)
             "ht8" [10, 128, 2048] fp8e4  (fp8 pieces)
             piece p, partition r, col c*512+f = hidden[(4p+c)*128 + r, f]
             "w16"/"w8" [128, 1024]       (w[r, k*16+q] = 1 iff row r of
                                           chunk k has class q)
    Output:  "ob" [128, 512] f32          (partitions 32g..32g+16 = group g:
                                           0 h-sums even chunks, 1 h-sums
                                           odd, 2 sq-sums even, 3 sq odd)
    """
    import concourse.tile as tile
    from concourse import bacc, mybir

    f32 = mybir.dt.float32
    f16 = mybir.dt.float16
    f8 = mybir.dt.float8e4

    nc = bacc.Bacc("TRN2", target_bir_lowering=False, debug=False,
                   num_devices=NCORES)
    ht16 = nc.declare_dram_parameter("ht16", [len(DVE_FP16), P, PIECE_COLS],
                                     f16, isOutput=False)
    ht8 = nc.declare_dram_parameter("ht8", [len(FP8_PIECES), P, PIECE_COLS],
                                    f8, isOutput=False)
    w16 = nc.declare_dram_parameter("w16", [P, CHUNKS * C], f16,
                                    isOutput=False)
    w8 = nc.declare_dram_parameter("w8", [P, CHUNKS * C], f8, isOutput=False)
    ob = nc.declare_dram_parameter("ob", [P, D], f32, isOutput=True)

    slot16 = {p: i for i, p in enumerate(DVE_FP16)}
    slot8 = {p: i for i, p in enumerate(FP8_PIECES)}

    with tile.TileContext(nc) as tc:
        with (
            tc.tile_pool(name="pc16", bufs=4) as pool16,
            tc.tile_pool(name="pc8", bufs=6) as pool8,
            tc.tile_pool(name="sq", bufs=4) as sq_pool,
            tc.tile_pool(name="wp", bufs=1) as w_pool,
            tc.psum_pool(name="ps", bufs=1) as psum_pool,
        ):
            # hoist the ACT table load to the head of the program
            dummy = w_pool.tile([P, 8], f16, tag="dummy")
            nc.gpsimd.memset(dummy[:], 0)
            nc.scalar.square(dummy[:], dummy[:])

            wsb16 = w_pool.tile([P, CHUNKS * C], f16, tag="wsb16")
            nc.sync.dma_start(wsb16[:], w16[:])
            wsb8 = w_pool.tile([P, CHUNKS * C], f8, tag="wsb8")
            nc.sync.dma_start(wsb8[:], w8[:])

            banks = [psum_pool.tile([P, D], f32, name=f"acc{g}",
                                    tag=f"acc{g}")
                     for g in range(4)]
            accs = [banks[g][32 * g:32 * g + C, :] for g in range(4)]
            started = [False] * 4
            n_issued = [0] * 4

            def seg_mm(g, k, wsb, rhs):
                first = not started[g]
                started[g] = True
                n_issued[g] += 1
                last = n_issued[g] == CHUNKS // 2
                nc.tensor.matmul(
                    accs[g], wsb[:, k * C:(k + 1) * C], rhs,
                    start=first, stop=last, tile_position=(0, 32 * g))

            sq_tiles = [None] * NPIECE

            def issue_sq_mms(p):
                t = sq_tiles[p]
                for c in range(CPP):
                    k = p * CPP + c
                    seg_mm(2 + (k % 2), k, wsb16, t[:, c * D:(c + 1) * D])

            for p in range(NPIECE):
                if p in FP16_PIECES:
                    t = pool16.tile([P, PIECE_COLS], f16, name=f"t16_{p}",
                                    tag="t16")
                    nc.sync.dma_start(t[:], ht16[slot16[p]])
                    hw = wsb16
                else:
                    t = pool8.tile([P, PIECE_COLS], f8, name=f"t8_{p}",
                                   tag="t8")
                    nc.gpsimd.dma_start(t[:], ht8[slot8[p]])
                    hw = wsb8

                sq = sq_pool.tile([P, PIECE_COLS], f16, name=f"sq_{p}",
                                  tag="sq")
                if p in ACT_FP8:
                    nc.scalar.square(sq[:], t[:])
                else:
                    nc.vector.tensor_mul(sq[:], t[:], t[:])
                sq_tiles[p] = sq

                # h matmuls for piece p; sq matmuls for piece p-1 (ready)
                for c in range(CPP):
                    k = p * CPP + c
                    seg_mm(k % 2, k, hw, t[:, c * D:(c + 1) * D])
                if p > 0:
                    issue_sq_mms(p - 1)
            issue_sq_mms(NPIECE - 1)

            ob_sb = w_pool.tile([P, D], f32, tag="ob_sb")
            for g in range(4):
                if g % 2 == 0:
                    nc.vector.tensor_copy(ob_sb[32 * g:32 * g + C, :], accs[g])
                else:
                    nc.scalar.copy(ob_sb[32 * g:32 * g + C, :], accs[g])
            nc.sync.dma_start(ob[:], ob_sb[:])
    nc.compile()
    return nc


def _get_nc():
    if "nc" not in _NC_CACHE:
        _NC_CACHE["nc"] = _build_nc()
    return _NC_CACHE["nc"]


def _prep_core(h_k, ids_k):
    import ml_dtypes

    pieces = h_k.reshape(NPIECE, CPP, P, D).transpose(0, 2, 1, 3)
    T16 = np.ascontiguousarray(
        pieces[list(DVE_FP16)].astype(np.float16)
    ).reshape(len(DVE_FP16), P, PIECE_COLS)
    T8 = np.ascontiguousarray(
        pieces[list(FP8_PIECES)].astype(ml_dtypes.float8_e4m3)
    ).reshape(len(FP8_PIECES), P, PIECE_COLS)

    ids2 = ids_k.reshape(CHUNKS, P)
    W3 = np.zeros((P, CHUNKS, C), dtype=np.float16)
    k_idx = np.broadcast_to(np.arange(CHUNKS)[:, None], (CHUNKS, P))
    r_idx = np.broadcast_to(np.arange(P)[None, :], (CHUNKS, P))
    W3[r_idx, k_idx, ids2] = 1.0
    W16 = W3.reshape(P, CHUNKS * C)
    return T16, T8, W16, W16.astype(ml_dtypes.float8_e4m3)


def _device_stats(hidden, ids, **run_kwargs):
    """Returns (sums[C,D], sumsq[C,D]) float64, plus the raw run result."""
    from concourse import bass_utils

    nc = _get_nc()

    in_maps = []
    for k in range(NCORES):
        rows = slice(k * ROWS, (k + 1) * ROWS)
        T16, T8, W16, W8 = _prep_core(hidden[rows], ids[rows])
        in_maps.append({"ht16": T16, "ht8": T8, "w16": W16, "w8": W8})

    res = bass_utils.run_bass_kernel_spmd(nc, in_maps, list(range(NCORES)),
                                          **run_kwargs)

    sums = np.zeros((C, D), dtype=np.float64)
    sumsq = np.zeros((C, D), dtype=np.float64)
    for k in range(NCORES):
        ob = res.results[k]["ob"].astype(np.float64)
        sums += ob[0:C] + ob[32:32 + C]
        sumsq += ob[64:64 + C] + ob[96:96 + C]
    return sums, sumsq, res


def _pairwise_loss(counts, sums, sumsq, d):
    """The tiny O(C^2 D) stage on host CPU.

    Runs in float32 with the same jax ops as the reference: at these extreme
    betainc parameters (b ~ 8190, x ~ 1e-5) jax's f32 betainc differs from
    the true (f64) value by ~1e-3, so matching the reference requires
    replicating its f32 numerics, not improving on them.
    """
    import jax
    import jax.numpy as jnp

    cpu = jax.devices("cpu")[0]
    with jax.default_device(cpu):
        counts64 = counts.astype(np.float64)
        means64 = sums / counts64[:, None]
        withins64 = sumsq - counts64[:, None] * means64**2
        counts = jnp.asarray(counts64, jnp.float32)               # [C]
        means = jnp.asarray(means64, jnp.float32)                 # [C, D]
        withins = jnp.asarray(withins64, jnp.float32)             # [C, D]
        half_diff = (means[:, None, :] - means[None, :, :]) * 0.5
        pair_counts = counts[:, None] + counts[None, :]
        pair_between = half_diff * half_diff * pair_counts[:, :, None]
        pair_within = withins[:, None, :] + withins[None, :, :]
        d2 = pair_counts - 2.0
        d2 = jnp.where(d2 == 0.0, 1e-5, d2)
        x = pair_between / (pair_between + pair_within)
        x = jnp.clip(x, XMIN, XMAX)
        a = jnp.full_like(x, 0.5)
        b = jnp.broadcast_to((d2 * 0.5)[:, :, None], x.shape)
        xbetainc = jax.scipy.special.betainc(a, b, x)             # [C, C, D]
        top_k, _ = jax.lax.top_k(xbetainc, int(d))                # [C, C, d]
        per_pair = jnp.sum(jnp.log(top_k), axis=-1)               # [C, C]
        mask = jnp.triu(jnp.ones((C, C), dtype=bool), k=1)
        total = jnp.sum(jnp.where(mask, per_pair, jnp.zeros_like(per_pair)))
        return float(-total)


def kernel(hidden, batch_ids, d):
    hidden = np.asarray(hidden, dtype=np.float32)
    ids = np.asarray(batch_ids).astype(np.int64)
    assert hidden.shape == (N, D), hidden.shape

    counts = np.bincount(ids, minlength=C).astype(np.float64)
    sums, sumsq, _ = _device_stats(hidden, ids)
    total = _pairwise_loss(counts, sums, sumsq, int(np.asarray(d)))
    return np.array(total, dtype=np.float32)
